# revision 8
# baseline (speedup 1.0000x reference)
"""Trainium2 Bass kernel for Gaussian KDE on a separable 2D grid.

out[b,i,j] = (1/Z_b) * sum_n exp(-||s_bn - (g_i, g_j)||^2 / (2h^2))

The evaluation grid is a meshgrid, so the Gaussian factorizes:
    exp(-((sx-g_i)^2 + (sy-g_j)^2)*inv) = fx[n,i] * fy[n,j]
    out_unnorm[b] = fx[b]^T @ fy[b]   (contraction over N=4096 on TensorE)

Sharding (8 cores): core c handles batch b = c % 4 and grid-row half
h = c // 4 (rows i in [64h, 64h+64)).  Each core computes fx for its 64
gx values and fy for all 128 gy values; halves are concatenated on the
host (pure unshard, no host math on outputs).

Normalization is FOLDED INTO THE OPERANDS: Z_b is computed on the host
from the inputs via the erf identity
    sum_i exp(-(s-g_i)^2/(2h^2)) = C*[Phi((hi-s)/h)-Phi((lo-s)/h)],
    C = h*sqrt(2*pi)/delta   (Poisson-summation error ~e^-88)
and ln(Z_b)/(2*inv) is added to both g^2 aug rows before the bf16 hi/lo
split, so exp(-inv*d'^2) = exp(-inv*d^2)/sqrt(Z_b) and the final PSUM
grid is already normalized.  This removes the on-device reduce /
reciprocal / broadcast / rescale chain entirely.

Per core:
  1. two parallel input DMAs: A = [g_aug (14,192) | sxy chunks 0-3],
     B = sxy chunks 4-31 (so the first aug matmul starts ~1us earlier)
  2. ACT exp-table prewarm on a dummy tile (hides the 1.3us table load)
  3. fused K=14 aug matmul per 128-sample chunk -> [dx'^2 | dy'^2]
     (128, 192) in PSUM, 4 chunks per PSUM tile
  4. ACT exp over (128, 768) PSUM -> SBUF f (bf16)
  5. big matmul accumulating over 32 chunks -> PSUM half-grid (64, 128)
  6. DVE copy PSUM->SBUF, DMA out 32KB.
"""

import numpy as np

B, N, H, W = 4, 4096, 128, 128
BANDWIDTH = 0.1
INV = 1.0 / (2.0 * BANDWIDTH * BANDWIDTH)  # 50.0
NCHUNK = N // 128  # 32
N_CORES = 8
K_AUG = 14  # 7 bf16 hi/lo-split rows per axis (see _prep_in_maps)
GCOLS = 192  # 64 gx + 128 gy columns per chunk
HH = H // 2  # 64 grid rows per core

_cache = {}


def _split_excess_waits(nc, max_waits=1):
    """walrus on this image rejects >1 sem wait per instruction
    ('Too many sync wait commands'); hoist excess waits onto NOPs."""
    import concourse.mybir as mybir

    ctr = 0
    for f in nc.m.functions:
        for blk in f.blocks:
            out = []
            changed = False
            for inst in blk.instructions:
                si = inst.sync_info
                if si is not None and len(si.on_wait) > max_waits:
                    waits = list(si.on_wait)
                    excess = waits[max_waits:]
                    for k in range(0, len(excess), max_waits):
                        ctr += 1
                        out.append(
                            mybir.InstNoOp(
                                name=f"{inst.name}-ws{ctr}",
                                sync_info=mybir.SyncInfo(
                                    on_wait=excess[k : k + max_waits], on_update=[]
                                ),
                                bass_nofuse=True,
                                engine=inst.engine,
                            )
                        )
                    inst.sync_info = mybir.SyncInfo(
                        on_wait=waits[:max_waits], on_update=list(si.on_update)
                    )
                    changed = True
                out.append(inst)
            if changed:
                blk.instructions = out


def _build():
    import concourse.bass as bass
    import concourse.mybir as mybir
    import concourse.tile as tile

    f32 = mybir.dt.float32
    bf16 = mybir.dt.bfloat16
    nc = bass.Bass("TRN2", target_bir_lowering=False, debug=False, num_devices=N_CORES)

    # A: [g_aug (192 cols) | sxy chunks 0-1]; B: chunks 2-13; C: chunks 14-31.
    # Three parallel DMAs (SP / Activation HWDGE + gpsimd SWDGE) so the first
    # aug matmul starts as early as possible and later chunks still arrive in
    # time.
    A_CH, B_CH = 2, 12
    A_COLS = GCOLS + A_CH * 128
    IN_A = nc.dram_tensor("in_a", [K_AUG, A_COLS], bf16, kind="ExternalInput")
    IN_B = nc.dram_tensor("in_b", [K_AUG, B_CH * 128], bf16, kind="ExternalInput")
    IN_C = nc.dram_tensor(
        "in_c", [K_AUG, (NCHUNK - A_CH - B_CH) * 128], bf16, kind="ExternalInput"
    )
    OUT = nc.dram_tensor("out", [HH, W], f32, kind="ExternalOutput")

    Exp = mybir.ActivationFunctionType.Exp

    with tile.TileContext(nc) as tc:
        with (
            tc.tile_pool(name="cst", bufs=1) as cst,
            tc.tile_pool(name="sb", bufs=1) as sb,
            tc.tile_pool(name="ps", bufs=3, space="PSUM") as ps,
            tc.tile_pool(name="pso", bufs=1, space="PSUM") as pso,
        ):
            a_sb = cst.tile([K_AUG, A_COLS], bf16, tag="ina")
            b_sb = cst.tile([K_AUG, B_CH * 128], bf16, tag="inb")
            c_sb = cst.tile([K_AUG, (NCHUNK - A_CH - B_CH) * 128], bf16, tag="inc")
            nc.sync.dma_start(a_sb[:], IN_A.ap()[:])
            nc.scalar.dma_start(b_sb[:], IN_B.ap()[:])
            nc.gpsimd.dma_start(c_sb[:], IN_C.ap()[:])

            # exp-table prewarm: runs while the input DMAs are in flight so
            # the 1.3us ACT_TABLE_LOAD is off the critical path.
            warm_in = cst.tile([128, 1], f32, tag="warm_i")
            warm_out = cst.tile([128, 1], f32, tag="warm_o")
            nc.vector.memset(warm_in[:], 0.0)
            nc.scalar.activation(warm_out[:], warm_in[:], Exp, bias=0.0, scale=1.0)

            # PSUM aug tiles use a 256-col stride per chunk (192 used + 64
            # pad) so every matmul write is 1KB-aligned and never straddles a
            # 2KB PSUM bank boundary (a straddling write misbehaves when
            # prior NEFFs leave stale bank state).  The exp activation reads
            # the PSUM through a strided 3D AP that SKIPS the pad, writing a
            # contiguous (128, 4, 192) f slab — so ACT pays 768 cols/group,
            # not 1024, and the Tensor engine stays the pacing engine.
            f_sb = sb.tile([128, NCHUNK, GCOLS], bf16, tag="f")

            for t in range(8):
                pf = ps.tile([128, 4, 256], f32, tag="aug")
                for q in range(4):
                    c = t * 4 + q
                    if c < A_CH:
                        lhs = a_sb[:, GCOLS + c * 128 : GCOLS + (c + 1) * 128]
                    elif c < A_CH + B_CH:
                        k = c - A_CH
                        lhs = b_sb[:, k * 128 : (k + 1) * 128]
                    else:
                        k = c - A_CH - B_CH
                        lhs = c_sb[:, k * 128 : (k + 1) * 128]
                    nc.tensor.matmul(
                        pf[:, q : q + 1, 0:GCOLS],
                        lhs,
                        a_sb[:, 0:GCOLS],
                        start=True,
                        stop=True,
                    )
                nc.scalar.activation(
                    f_sb[:, t * 4 : (t + 1) * 4, :],
                    pf[:, :, 0:GCOLS],
                    Exp,
                    bias=0.0,
                    scale=-INV,
                )

            # big matmul: out[i,j] = sum_n fx[n,i] fy[n,j], PSUM-accumulated;
            # already normalized thanks to the folded ln(Z) terms.
            po = pso.tile([HH, W], f32, tag="out")
            for c in range(NCHUNK):
                nc.tensor.matmul(
                    po[:],
                    f_sb[:, c : c + 1, 0:HH],
                    f_sb[:, c : c + 1, HH:GCOLS],
                    start=(c == 0),
                    stop=(c == NCHUNK - 1),
                )

            out_sb = sb.tile([HH, W], f32, tag="outsb")
            nc.vector.tensor_copy(out_sb[:], po[:])
            nc.sync.dma_start(OUT.ap()[:], out_sb[:])

    _split_excess_waits(nc)
    return nc


def _split_bf16(x):
    """x (fp32) -> (hi, lo) bf16 with hi + lo ~= x to ~16 mantissa bits."""
    import ml_dtypes

    hi = x.astype(ml_dtypes.bfloat16)
    lo = (x - hi.astype(np.float32)).astype(ml_dtypes.bfloat16)
    return hi, lo


def _axis_F(s, g):
    """sum_i exp(-INV*(s-g_i)^2) for a uniform grid g, via the erf identity
    (Poisson-summation residual ~e^-88 at h/delta ~ 2.1). float64 in/out."""
    from math import erf, pi, sqrt

    delta = (g[-1] - g[0]) / (len(g) - 1)
    C = BANDWIDTH * sqrt(2 * pi) / delta
    lo = g[0] - delta / 2
    hi = g[-1] + delta / 2
    r = BANDWIDTH * sqrt(2)
    try:
        from scipy.special import erf as verf

        return C * 0.5 * (verf((hi - s) / r) - verf((lo - s) / r))
    except ImportError:
        return np.array(
            [C * 0.5 * (erf((hi - v) / r) - erf((lo - v) / r)) for v in s], np.float64
        )


def _g_aug(gx64, gy, cz):
    """(14, 192) bf16 aug rhs; cz = ln(Z)/(2*INV) folded into both g^2 rows."""
    import ml_dtypes

    bf16 = ml_dtypes.bfloat16
    g = np.zeros((K_AUG, GCOLS), bf16)
    for ax, (gv, cols) in enumerate(((gx64, slice(0, HH)), (gy, slice(HH, GCOLS)))):
        g2h, g2l = _split_bf16((gv * gv + cz).astype(np.float32))
        gh, gl = _split_bf16((-2.0 * gv).astype(np.float32))
        r = 7 * ax
        g[r + 0, cols] = g2h
        g[r + 1, cols] = g2l
        g[r + 2, cols] = 1.0
        g[r + 3, cols] = 1.0
        g[r + 4, cols] = gh
        g[r + 5, cols] = gl
        g[r + 6, cols] = gh
    return g


def _sxy_aug(samples_b):
    """(14, 4096) bf16 aug lhs for one batch element."""
    import ml_dtypes

    bf16 = ml_dtypes.bfloat16
    sxy = np.zeros((K_AUG, N), bf16)
    for ax in range(2):
        s = samples_b[:, ax].astype(np.float32)
        s2h, s2l = _split_bf16(s * s)
        sh, sl = _split_bf16(s)
        r = 7 * ax
        sxy[r + 0] = 1.0
        sxy[r + 1] = 1.0
        sxy[r + 2] = s2h
        sxy[r + 3] = s2l
        sxy[r + 4] = sh
        sxy[r + 5] = sh
        sxy[r + 6] = sl
    return sxy


def _prep_in_maps(samples, locations):
    samples = np.asarray(samples, np.float32)
    locations = np.asarray(locations, np.float32)
    gi = np.ascontiguousarray(locations[:, 0, 0]).astype(np.float64)
    gj = np.ascontiguousarray(locations[0, :, 1]).astype(np.float64)

    sxys = []
    czs = []
    for b in range(B):
        sx = samples[b, :, 0].astype(np.float64)
        sy = samples[b, :, 1].astype(np.float64)
        Z = float((_axis_F(sx, gi) * _axis_F(sy, gj)).sum())
        czs.append(np.float32(np.log(Z) / (2.0 * INV)))
        sxys.append(_sxy_aug(samples[b]))

    in_maps = []
    for c in range(N_CORES):
        b, half = c % B, c // B
        gx64 = gi[half * HH : (half + 1) * HH].astype(np.float32)
        g = _g_aug(gx64, gj.astype(np.float32), czs[b])
        sxy = sxys[b]
        in_a = np.concatenate([g, sxy[:, : 2 * 128]], axis=1)
        in_b = np.ascontiguousarray(sxy[:, 2 * 128 : 14 * 128])
        in_c = np.ascontiguousarray(sxy[:, 14 * 128 :])
        in_maps.append({"in_a": in_a, "in_b": in_b, "in_c": in_c})
    return in_maps


def _assemble(results):
    out = np.empty((B, H, W), np.float32)
    for c in range(N_CORES):
        b, half = c % B, c // B
        out[b, half * HH : (half + 1) * HH, :] = results[c]["out"]
    return out


def kernel(samples: np.ndarray, locations: np.ndarray) -> np.ndarray:
    from concourse.bass_utils import run_bass_kernel_spmd

    if "nc" not in _cache:
        _cache["nc"] = _build()
    nc = _cache["nc"]

    in_maps = _prep_in_maps(samples, locations)
    res = run_bass_kernel_spmd(nc, in_maps, core_ids=list(range(N_CORES)))
    return _assemble(res.results)
